# revision 28
# baseline (speedup 1.0000x reference)
"""TopoEncoder Trainium2 kernel v4 (8 NeuronCores, data-parallel over batch).

Changes vs v3 (90.6-115us):
  - x arrives host-pretransposed in the exact SBUF tile layout
    ([2, 128, C, 32, V]) so the load is 4 DMAs of 64 contiguous 9.6KB
    rows instead of ~1.5k small descriptors: add tree starts ~7us
    earlier, CC trigger ~22us instead of ~32us.
  - Floyd-Warshall min-max closure runs on d^2 (monotone transform
    keeps the closure/mask identical), so the big [B,625] sqrt is gone;
    deaths are sqrt'd after extraction on [B,24] only.
  - The collective gathers the local max of d (sqrt'd pre-CC), so the
    post-CC path needs no scalar-engine table: vector max + reciprocal
    + PE broadcast only. Scalar act tables (SquareExp -> Sqrt ->
    SquareExp) all load inside the FW window via data-pinned warm ops.
  - deaths replicate matmul runs pre-CC; extraction no longer stalls
    behind a CC-dependent vector op (the pin bypass now depends on the
    final extraction round).
  - Output assembled to [64,64] PSUM via two selector matmuls -> one
    DMA, instead of two strided SBUF DMAs.
"""

from contextlib import ExitStack

import numpy as np

import bass_rust
import concourse.bass as bass
import concourse.tile as tile
from concourse import mybir
from concourse.bass_utils import run_bass_kernel_spmd

N_CORES = 8
B = 64          # samples per core
C, T, V, E = 3, 128, 25, 64
VV = V * V
NT = V - 1      # deaths per sample (24)
E2 = E // 2     # e-half per partition group (32)
DT = mybir.dt.float32


def _split_excess_waits(nc, cap=1):
    """The walrus build in this env rejects instructions carrying more than
    ~2 semaphore-wait commands. Move excess waits onto same-engine NOPs
    inserted immediately before the offending instruction."""
    n_split = 0
    for bb in nc.main_func.blocks:
        insts = bb.instructions
        i = 0
        while i < len(insts):
            ins = insts[i]
            si = ins.sync_info
            waits = list(si.on_wait) if si and si.on_wait else []
            if len(waits) > cap:
                extra, keep = waits[:-cap], waits[-cap:]
                ins.sync_info = mybir.SyncInfo(
                    on_wait=keep, on_update=list(si.on_update or [])
                )
                for j, w in enumerate(extra):
                    nop = bass_rust.InstNoOp(
                        name=f"I-wsplit-{n_split}-{j}",
                        engine=ins.engine,
                        sync_info=mybir.SyncInfo(on_wait=[w], on_update=[]),
                    )
                    insts.insert(i, nop)
                    i += 1
                n_split += 1
            i += 1
    return n_split


def _build_program(cc_groups=None):
    A = mybir.AluOpType
    ACT = mybir.ActivationFunctionType
    nc = bass.Bass("TRN2", debug=False, num_devices=N_CORES)
    if cc_groups is None:
        cc_groups = [list(range(N_CORES))]
    n_gather = len(cc_groups[0])

    # x pre-transposed on host: [half, partition, c, v, t32] (t innermost
    # so the T-reduction is a single axis=X tensor_reduce per engine slice)
    x_in = nc.dram_tensor("x", [2, 128, C, V, 32], DT, kind="ExternalInput").ap()
    # csH rows: h -> [c2_h(32) | s2_h(32) | s1_h(32) | c1_h(32)]
    csH_in = nc.dram_tensor("csH", [2, 4 * E2], DT, kind="ExternalInput").ap()
    pm_in = nc.dram_tensor("pm", [128, B], DT, kind="ExternalInput").ap()
    ut_in = nc.dram_tensor("ut", [1, VV], DT, kind="ExternalInput").ap()
    rep_in = nc.dram_tensor("rep", [B, 128], DT, kind="ExternalInput").ap()
    sel_in = nc.dram_tensor("sel", [2, 128], DT, kind="ExternalInput").ap()
    out_d = nc.dram_tensor("out", [B, E], DT, kind="ExternalOutput").ap()

    with tile.TileContext(nc, num_cores=N_CORES) as tc, ExitStack() as ctx:
        sb = ctx.enter_context(tc.tile_pool(name="sb", bufs=1))
        work = ctx.enter_context(tc.tile_pool(name="work", bufs=2))
        psum = ctx.enter_context(tc.tile_pool(name="psum", bufs=1, space="PSUM"))
        dram = ctx.enter_context(tc.tile_pool(name="dram", bufs=1, space="DRAM"))

        ones1 = sb.tile([1, 128], DT)
        nc.vector.memset(ones1[:], 1.0)

        # ---- x DMA on scalar in c01/c2 chunks (reduces start per chunk);
        # consts on sync so PE's inputs are not queued behind the x bulk
        xa = sb.tile([128, C, V, T // 4], DT)
        xb = sb.tile([128, C, V, T // 4], DT)
        nc.scalar.dma_start(xa[:, 0:2], x_in[0, :, 0:2])
        nc.scalar.dma_start(xa[:, 2], x_in[0, :, 2])
        nc.scalar.dma_start(xb[:, 0:2], x_in[1, :, 0:2])
        nc.scalar.dma_start(xb[:, 2], x_in[1, :, 2])

        # ---- small constant loads ----
        pm_t = sb.tile([128, B], DT)
        nc.sync.dma_start(pm_t[:], pm_in[:])
        csh = sb.tile([2, 4 * E2], DT)
        nc.sync.dma_start(csh[:], csH_in[:])
        utrow = sb.tile([1, VV], DT)
        nc.sync.dma_start(utrow[:], ut_in[:])
        rep128 = sb.tile([B, 128], DT)
        nc.sync.dma_start(rep128[:], rep_in[:])
        sel2 = sb.tile([2, 128], DT)
        nc.sync.dma_start(sel2[:], sel_in[:])
        lmx = sb.tile([1, 1], DT)
        lmxd = sb.tile([1, 1], DT)
        gsb = sb.tile([1, n_gather], DT)

        # ---- PE partition-broadcasts ----
        utb = psum.tile([B, VV], DT)
        nc.tensor.matmul(out=utb[:, 0:512], lhsT=ones1[:, 0:B],
                         rhs=utrow[:, 0:512], start=True, stop=True)
        nc.tensor.matmul(out=utb[:, 512:VV], lhsT=ones1[:, 0:B],
                         rhs=utrow[:, 512:VV], start=True, stop=True)
        # GpSimd cannot read PSUM: bounce the upper-tri broadcast to SBUF
        # early, while the vector queue is idle
        utsb = sb.tile([B, VV], DT)
        nc.vector.tensor_copy(utsb[:], utb[:])
        # per-half params onto 128 partitions: partition p gets half p//64
        prm2 = psum.tile([128, 4, E2], DT)
        nc.tensor.matmul(out=prm2[:], lhsT=sel2[:], rhs=csh[:],
                         start=True, stop=True)
        prm = sb.tile([128, 4, E2], DT)
        nc.vector.tensor_copy(prm[:], prm2[:])
        c2p = prm[:, 0, :]

        # derived parameter tiles: ub = s2^2, Ab = exp(-(s1*c1)^2)
        ub = sb.tile([128, E2], DT)
        nc.scalar.square(ub[:], prm[:, 1, :])
        m1t = sb.tile([128, E2], DT)
        nc.vector.tensor_tensor(out=m1t[:], in0=prm[:, 2, :], in1=prm[:, 3, :],
                                op=A.mult)
        nc.scalar.square(m1t[:], m1t[:])
        Ab = sb.tile([128, E2], DT)
        nc.scalar.activation(Ab[:], m1t[:], ACT.Exp, bias=0.0, scale=-1.0)

        # ---- mean over T: single axis=X reductions (DVE: c0-c1 + part of
        # c2, GpSimd: rest of c2 -- balanced by engine rates), cross-half
        # add, then PE pair-matrix fold ----
        ra = sb.tile([128, C, V], DT)
        rb = sb.tile([128, C, V], DT)
        for xh, rh in ((xa, ra), (xb, rb)):
            nc.vector.tensor_reduce(out=rh[:, 0:2], in_=xh[:, 0:2],
                                    axis=mybir.AxisListType.X, op=A.add)
            nc.vector.tensor_reduce(out=rh[:, 2], in_=xh[:, 2],
                                    axis=mybir.AxisListType.X, op=A.add)
        # cross-half add folded into the PE pair fold via PSUM accumulation
        ps_xm = psum.tile([B, C, V], DT)
        nc.tensor.matmul(out=ps_xm[:], lhsT=pm_t[:], rhs=ra[:],
                         start=True, stop=False)
        nc.tensor.matmul(out=ps_xm[:], lhsT=pm_t[:], rhs=rb[:],
                         start=False, stop=True)
        xm = sb.tile([B, C, V], DT)
        nc.vector.tensor_copy(xm[:], ps_xm[:])

        # ---- squared distance matrix (no sqrt: FW closure is monotone);
        # all subtracts on V (GpSimd is 3x slower and gated the adds) ----
        df = sb.tile([B, C, V, V], DT)
        xmb_i = xm.unsqueeze(-1).broadcast_to([B, C, V, V])
        xmb_j = xm.unsqueeze(2).broadcast_to([B, C, V, V])
        nc.vector.tensor_tensor(
            out=df[:, 0:2], in0=xmb_i[:, 0:2], in1=xmb_j[:, 0:2], op=A.subtract
        )
        nc.vector.tensor_tensor(
            out=df[:, 2], in0=xmb_i[:, 2], in1=xmb_j[:, 2], op=A.subtract
        )
        nc.scalar.square(df[:, 0:2], df[:, 0:2])
        nc.vector.tensor_tensor(out=df[:, 2], in0=df[:, 2], in1=df[:, 2], op=A.mult)
        d2 = sb.tile([B, VV], DT)
        d23 = d2.rearrange("p (i j) -> p i j", i=V)
        nc.vector.tensor_tensor(out=d23[:], in0=df[:, 0], in1=df[:, 1], op=A.add)
        nc.vector.tensor_tensor(out=d23[:], in0=d23[:], in1=df[:, 2], op=A.add)

        # ---- local max(d2) -> sqrt -> AllGather of local max(d) ----
        nc.gpsimd.tensor_reduce(out=lmx[:], in_=d2[:],
                                axis=mybir.AxisListType.XYZWC, op=A.max)
        # Sqrt table load is eager at the scalar queue head (~22us); both
        # sqrts (this one and the deaths one) use it.
        nc.scalar.sqrt(lmxd[:], lmx[:])
        cin = dram.tile([1, 1], DT)
        cout = dram.tile([1, n_gather], DT)
        nc.scalar.dma_start(cin[:], lmxd[:])
        nc.gpsimd.collective_compute(
            "AllGather", A.bypass, replica_groups=cc_groups,
            ins=[cin.opt()], outs=[cout.opt()],
        )
        nc.sync.dma_start(gsb[:], cout[:])

        # ---- premasked values: on GpSimd (mult is Pool-legal), under the
        # FW window ----
        dut = sb.tile([B, VV], DT)
        nc.gpsimd.tensor_tensor(out=dut[:], in0=d2[:], in1=utsb[:], op=A.mult)

        # ---- Floyd-Warshall min-max closure on d2 (vector only: the Pool
        # engine's TensorTensor has no min/max/is_ge) ----
        M = sb.tile([B, VV], DT)
        M3 = M.rearrange("p (i j) -> p i j", i=V)
        fwt = sb.tile([B, V, V], DT)
        for k in range(V):
            src = d23 if k == 0 else M3
            nc.vector.tensor_tensor(
                out=fwt[:],
                in0=src[:, :, k : k + 1].broadcast_to([B, V, V]),
                in1=src[:, k : k + 1, :].broadcast_to([B, V, V]),
                op=A.max,
            )
            nc.vector.tensor_tensor(out=M3[:], in0=src[:], in1=fwt[:], op=A.min)

        # ---- MST mask + masked upper-tri values (d2 scale) ----
        mk = sb.tile([B, VV], DT)
        nc.vector.tensor_tensor(out=mk[:], in0=M[:], in1=d2[:], op=A.is_ge)
        val = sb.tile([B, VV], DT)
        nc.vector.tensor_tensor(out=val[:], in0=mk[:], in1=dut[:], op=A.mult)

        # ---- extract 24 MST weights^2: 3 rounds of top-8 + match_replace ----
        dsq = sb.tile([B, NT], DT)
        mr1 = sb.tile([B, VV], DT)
        mr2 = sb.tile([B, VV], DT)
        nc.vector.max(dsq[:, 0:8], val[:])
        nc.vector.match_replace(mr1[:], dsq[:, 0:8], val[:], 0.0)
        nc.vector.max(dsq[:, 8:16], mr1[:])
        nc.vector.match_replace(mr2[:], dsq[:, 8:16], mr1[:], 0.0)
        nc.vector.max(dsq[:, 16:24], mr2[:])

        # deaths = sqrt(deaths^2): tiny, Sqrt table already resident
        deaths = sb.tile([B, NT], DT)
        nc.scalar.sqrt(deaths[:], dsq[:])
        # warm ops pinned after the sqrt: pull the SquareExp table reload
        # into the FW/CC window so the tail pays no table switch
        wtmp = sb.tile([1, 2], DT)
        nc.scalar.square(wtmp[:, 0:1], deaths[0:1, 0:1])
        nc.scalar.activation(wtmp[:, 1:2], deaths[0:1, 0:1], ACT.Exp,
                             bias=0.0, scale=-1.0)

        # ---- replicate deaths onto both partition halves (pre-CC) ----
        deaths2 = psum.tile([128, NT], DT)
        nc.tensor.matmul(out=deaths2[:], lhsT=rep128[:], rhs=deaths[:],
                         start=True, stop=True)

        # ---- post-CC: global max -> inv -> normalize ----
        # pin: first post-CC vector op also depends on the last extraction
        # round so the scheduler cannot hoist it into the FW stream
        g8 = sb.tile([1, n_gather], DT)
        nc.vector.tensor_tensor(out=g8[:], in0=gsb[:],
                                in1=dsq[0:1, 16 : 16 + n_gather],
                                op=A.bypass)
        gmx = sb.tile([1, 1], DT)
        nc.vector.tensor_reduce(out=gmx[:], in_=g8[:],
                                axis=mybir.AxisListType.X, op=A.max)
        inv = sb.tile([1, 1], DT)
        nc.vector.reciprocal(inv[:], gmx[:])
        invb = psum.tile([128, 1], DT)
        nc.tensor.matmul(out=invb[:], lhsT=ones1[:], rhs=inv[:],
                         start=True, stop=True)
        dn = sb.tile([128, NT], DT)
        nc.vector.tensor_scalar_mul(dn[:], deaths2[:], invb[:, 0:1])

        # ---- structure element layer on 128 partitions (e-half per h) ----
        S = sb.tile([128, E2], DT)
        ECH = 16
        for ch in range(E2 // ECH):
            e0 = ch * ECH
            t1 = work.tile([128, ECH, NT], DT, tag="t1")
            nc.vector.tensor_tensor(
                out=t1[:],
                in0=dn.unsqueeze(1).broadcast_to([128, ECH, NT]),
                in1=c2p[:, e0 : e0 + ECH].unsqueeze(-1).broadcast_to([128, ECH, NT]),
                op=A.subtract,
            )
            nc.scalar.square(t1[:], t1[:])
            nc.vector.tensor_tensor(
                out=t1[:],
                in0=t1[:],
                in1=ub[:, e0 : e0 + ECH].unsqueeze(-1).broadcast_to([128, ECH, NT]),
                op=A.mult,
            )
            fexp = work.tile([128, ECH, NT], DT, tag="fexp")
            nc.scalar.activation(fexp[:], t1[:], ACT.Exp, bias=0.0, scale=-1.0)
            nc.vector.tensor_reduce(
                out=S[:, e0 : e0 + ECH], in_=fexp[:], axis=mybir.AxisListType.X,
                op=A.add,
            )
        outt = sb.tile([128, E2], DT)
        nc.vector.tensor_tensor(out=outt[:], in0=S[:], in1=Ab[:], op=A.mult)

        # ---- output: two direct DMAs, triggers on different engines so
        # the DIRECT2D processing overlaps ----
        nc.sync.dma_start(out_d[:, 0:E2], outt[0:B])
        nc.scalar.dma_start(out_d[:, E2:E], outt[B:128])

    _split_excess_waits(nc)
    return nc


_CACHE = {}


def _consts():
    # pair matrix: adds partition rows b and b+64 (the two T-halves) and
    # applies the 1/T mean scale
    pairmat = np.zeros((128, B), dtype=np.float32)
    for p in range(128):
        pairmat[p, p % B] = 1.0 / T
    ut = np.triu(np.ones((V, V), dtype=np.float32), k=1).reshape(1, VV)
    rep = np.zeros((B, 128), dtype=np.float32)
    for p in range(128):
        rep[p % B, p] = 1.0
    sel = np.zeros((2, 128), dtype=np.float32)
    sel[0, 0:B] = 1.0
    sel[1, B:128] = 1.0
    return pairmat, np.ascontiguousarray(ut), rep, sel


def _get_program(cc_groups=None, key="nc"):
    if key not in _CACHE:
        _CACHE[key] = _build_program(cc_groups)
    return _CACHE[key]


def _run(x, centres, sharpness, cc_groups=None, key="nc", **run_kwargs):
    nc = _get_program(cc_groups, key)
    xf = np.ascontiguousarray(x.reshape(-1, C, T, V)).astype(np.float32, copy=False)
    n_total = xf.shape[0]
    assert n_total == N_CORES * B, xf.shape
    c1, c2 = centres[:, 0], centres[:, 1]
    s1, s2 = sharpness[:, 0], sharpness[:, 1]
    csH = np.stack(
        [
            np.concatenate([c2[h * E2 : (h + 1) * E2], s2[h * E2 : (h + 1) * E2],
                            s1[h * E2 : (h + 1) * E2], c1[h * E2 : (h + 1) * E2]])
            for h in range(2)
        ],
        axis=0,
    ).astype(np.float32)
    pairmat, ut, rep, sel = _consts()
    in_maps = []
    for i in range(N_CORES):
        xc = xf[i * B : (i + 1) * B]  # [64, C, T, V]
        xt = xc.transpose(0, 1, 3, 2)  # [64, C, V, T]
        xh = np.empty((2, 128, C, V, 32), dtype=np.float32)
        xh[0, 0:64] = xt[..., 0:32]
        xh[0, 64:128] = xt[..., 64:96]
        xh[1, 0:64] = xt[..., 32:64]
        xh[1, 64:128] = xt[..., 96:128]
        in_maps.append(
            {
                "x": np.ascontiguousarray(xh),
                "csH": np.ascontiguousarray(csH),
                "pm": pairmat,
                "ut": ut,
                "rep": rep,
                "sel": sel,
            }
        )
    res = run_bass_kernel_spmd(nc, in_maps, list(range(N_CORES)), **run_kwargs)
    out = np.concatenate([res.results[i]["out"] for i in range(N_CORES)], axis=0)
    return out, res


def kernel(x, centres, sharpness):
    out, _ = _run(np.asarray(x), np.asarray(centres), np.asarray(sharpness))
    return out


# revision 29
# speedup vs baseline: 1.0297x; 1.0297x over previous
"""TopoEncoder Trainium2 kernel v9 (8 NeuronCores, data-parallel over batch).

Measured 87.2-90.2us (baseline v3: 90.6-115us); exec = max(compute
~76.4us, CC end 75-100us) + ~10us tail. Key structure:
  - x host-pretransposed to [2, 128, C, V, t32] (t innermost): the
    T-mean is one axis=X tensor_reduce per c-chunk, and the load is 4
    contiguous-row DMAs on the scalar queue in c01/c2 chunks so each
    reduce starts as its chunk lands. Consts ride the sync queue so
    PE's broadcast inputs are not queued behind the x bulk.
  - Floyd-Warshall min-max closure on d^2 (monotone transform keeps
    closure/mask identical): no [B,625] sqrt; deaths sqrt'd on [B,24]
    after extraction. FW is vector-only: the Pool engine's TensorTensor
    ALU has no min/max/is_ge (arithmetic ops only), so GpSimd instead
    carries the upper-tri premask mult and the local-max reduce.
  - Cross-half add of the two T-halves folded into the PE pair fold
    via PSUM accumulation (two matmuls, start/stop split).
  - The collective AllGathers the local max of d (sqrt'd pre-CC inside
    the Sqrt-table window), so the post-CC path needs no act table:
    vector max + reciprocal + PE [1->128] broadcast only. Table loads
    (SquareExp -> Sqrt -> SquareExp) all hide in the FW window via
    data-pinned warm ops.
  - deaths-replicate matmul pre-CC; the post-CC pin bypass depends on
    the final extraction round so extraction never stalls behind the
    collective on the in-order vector queue.
  - Output: two direct DMAs (e-halves) with triggers on sync+scalar so
    the DIRECT2D processing overlaps.
"""

from contextlib import ExitStack

import numpy as np

import bass_rust
import concourse.bass as bass
import concourse.tile as tile
from concourse import mybir
from concourse.bass_utils import run_bass_kernel_spmd

N_CORES = 8
B = 64          # samples per core
C, T, V, E = 3, 128, 25, 64
VV = V * V
NT = V - 1      # deaths per sample (24)
E2 = E // 2     # e-half per partition group (32)
DT = mybir.dt.float32


def _split_excess_waits(nc, cap=1):
    """The walrus build in this env rejects instructions carrying more than
    ~2 semaphore-wait commands. Move excess waits onto same-engine NOPs
    inserted immediately before the offending instruction."""
    n_split = 0
    for bb in nc.main_func.blocks:
        insts = bb.instructions
        i = 0
        while i < len(insts):
            ins = insts[i]
            si = ins.sync_info
            waits = list(si.on_wait) if si and si.on_wait else []
            if len(waits) > cap:
                extra, keep = waits[:-cap], waits[-cap:]
                ins.sync_info = mybir.SyncInfo(
                    on_wait=keep, on_update=list(si.on_update or [])
                )
                for j, w in enumerate(extra):
                    nop = bass_rust.InstNoOp(
                        name=f"I-wsplit-{n_split}-{j}",
                        engine=ins.engine,
                        sync_info=mybir.SyncInfo(on_wait=[w], on_update=[]),
                    )
                    insts.insert(i, nop)
                    i += 1
                n_split += 1
            i += 1
    return n_split


def _build_program(cc_groups=None):
    A = mybir.AluOpType
    ACT = mybir.ActivationFunctionType
    nc = bass.Bass("TRN2", debug=False, num_devices=N_CORES)
    if cc_groups is None:
        cc_groups = [list(range(N_CORES))]
    n_gather = len(cc_groups[0])

    # x pre-transposed on host: [half, partition, c, v, t32] (t innermost
    # so the T-reduction is a single axis=X tensor_reduce per engine slice)
    x_in = nc.dram_tensor("x", [2, 128, C, V, 32], DT, kind="ExternalInput").ap()
    # csH rows: h -> [c2_h(32) | s2_h(32) | s1_h(32) | c1_h(32)]
    csH_in = nc.dram_tensor("csH", [2, 4 * E2], DT, kind="ExternalInput").ap()
    pm_in = nc.dram_tensor("pm", [128, B], DT, kind="ExternalInput").ap()
    ut_in = nc.dram_tensor("ut", [1, VV], DT, kind="ExternalInput").ap()
    rep_in = nc.dram_tensor("rep", [B, 128], DT, kind="ExternalInput").ap()
    sel_in = nc.dram_tensor("sel", [2, 128], DT, kind="ExternalInput").ap()
    out_d = nc.dram_tensor("out", [B, E], DT, kind="ExternalOutput").ap()

    with tile.TileContext(nc, num_cores=N_CORES) as tc, ExitStack() as ctx:
        sb = ctx.enter_context(tc.tile_pool(name="sb", bufs=1))
        work = ctx.enter_context(tc.tile_pool(name="work", bufs=2))
        psum = ctx.enter_context(tc.tile_pool(name="psum", bufs=1, space="PSUM"))
        dram = ctx.enter_context(tc.tile_pool(name="dram", bufs=1, space="DRAM"))

        ones1 = sb.tile([1, 128], DT)
        nc.vector.memset(ones1[:], 1.0)

        # ---- x DMA on scalar in c01/c2 chunks (reduces start per chunk);
        # consts on sync so PE's inputs are not queued behind the x bulk
        xa = sb.tile([128, C, V, T // 4], DT)
        xb = sb.tile([128, C, V, T // 4], DT)
        nc.scalar.dma_start(xa[:, 0:2], x_in[0, :, 0:2])
        nc.scalar.dma_start(xa[:, 2], x_in[0, :, 2])
        nc.scalar.dma_start(xb[:, 0:2], x_in[1, :, 0:2])
        nc.scalar.dma_start(xb[:, 2], x_in[1, :, 2])

        # ---- small constant loads ----
        pm_t = sb.tile([128, B], DT)
        nc.sync.dma_start(pm_t[:], pm_in[:])
        csh = sb.tile([2, 4 * E2], DT)
        nc.sync.dma_start(csh[:], csH_in[:])
        utrow = sb.tile([1, VV], DT)
        nc.sync.dma_start(utrow[:], ut_in[:])
        rep128 = sb.tile([B, 128], DT)
        nc.sync.dma_start(rep128[:], rep_in[:])
        sel2 = sb.tile([2, 128], DT)
        nc.sync.dma_start(sel2[:], sel_in[:])
        lmx = sb.tile([1, 1], DT)
        lmxd = sb.tile([1, 1], DT)
        gsb = sb.tile([1, n_gather], DT)

        # ---- PE partition-broadcasts ----
        utb = psum.tile([B, VV], DT)
        nc.tensor.matmul(out=utb[:, 0:512], lhsT=ones1[:, 0:B],
                         rhs=utrow[:, 0:512], start=True, stop=True)
        nc.tensor.matmul(out=utb[:, 512:VV], lhsT=ones1[:, 0:B],
                         rhs=utrow[:, 512:VV], start=True, stop=True)
        # GpSimd cannot read PSUM: bounce the upper-tri broadcast to SBUF
        # early, while the vector queue is idle
        utsb = sb.tile([B, VV], DT)
        nc.vector.tensor_copy(utsb[:], utb[:])
        # per-half params onto 128 partitions: partition p gets half p//64
        prm2 = psum.tile([128, 4, E2], DT)
        nc.tensor.matmul(out=prm2[:], lhsT=sel2[:], rhs=csh[:],
                         start=True, stop=True)
        prm = sb.tile([128, 4, E2], DT)
        nc.vector.tensor_copy(prm[:], prm2[:])
        c2p = prm[:, 0, :]

        # derived parameter tiles: ub = s2^2, Ab = exp(-(s1*c1)^2)
        ub = sb.tile([128, E2], DT)
        nc.scalar.square(ub[:], prm[:, 1, :])
        m1t = sb.tile([128, E2], DT)
        nc.vector.tensor_tensor(out=m1t[:], in0=prm[:, 2, :], in1=prm[:, 3, :],
                                op=A.mult)
        nc.scalar.square(m1t[:], m1t[:])
        Ab = sb.tile([128, E2], DT)
        nc.scalar.activation(Ab[:], m1t[:], ACT.Exp, bias=0.0, scale=-1.0)

        # ---- mean over T: single axis=X reductions (DVE: c0-c1 + part of
        # c2, GpSimd: rest of c2 -- balanced by engine rates), cross-half
        # add, then PE pair-matrix fold ----
        ra = sb.tile([128, C, V], DT)
        rb = sb.tile([128, C, V], DT)
        for xh, rh in ((xa, ra), (xb, rb)):
            nc.vector.tensor_reduce(out=rh[:, 0:2], in_=xh[:, 0:2],
                                    axis=mybir.AxisListType.X, op=A.add)
            nc.vector.tensor_reduce(out=rh[:, 2], in_=xh[:, 2],
                                    axis=mybir.AxisListType.X, op=A.add)
        # cross-half add folded into the PE pair fold via PSUM accumulation
        ps_xm = psum.tile([B, C, V], DT)
        nc.tensor.matmul(out=ps_xm[:], lhsT=pm_t[:], rhs=ra[:],
                         start=True, stop=False)
        nc.tensor.matmul(out=ps_xm[:], lhsT=pm_t[:], rhs=rb[:],
                         start=False, stop=True)
        xm = sb.tile([B, C, V], DT)
        nc.vector.tensor_copy(xm[:], ps_xm[:])

        # ---- squared distance matrix (no sqrt: FW closure is monotone);
        # all subtracts on V (GpSimd is 3x slower and gated the adds) ----
        df = sb.tile([B, C, V, V], DT)
        xmb_i = xm.unsqueeze(-1).broadcast_to([B, C, V, V])
        xmb_j = xm.unsqueeze(2).broadcast_to([B, C, V, V])
        nc.vector.tensor_tensor(
            out=df[:, 0:2], in0=xmb_i[:, 0:2], in1=xmb_j[:, 0:2], op=A.subtract
        )
        nc.vector.tensor_tensor(
            out=df[:, 2], in0=xmb_i[:, 2], in1=xmb_j[:, 2], op=A.subtract
        )
        nc.scalar.square(df[:, 0:2], df[:, 0:2])
        nc.vector.tensor_tensor(out=df[:, 2], in0=df[:, 2], in1=df[:, 2], op=A.mult)
        d2 = sb.tile([B, VV], DT)
        d23 = d2.rearrange("p (i j) -> p i j", i=V)
        nc.vector.tensor_tensor(out=d23[:], in0=df[:, 0], in1=df[:, 1], op=A.add)
        nc.vector.tensor_tensor(out=d23[:], in0=d23[:], in1=df[:, 2], op=A.add)

        # ---- local max(d2) -> sqrt -> AllGather of local max(d) ----
        nc.gpsimd.tensor_reduce(out=lmx[:], in_=d2[:],
                                axis=mybir.AxisListType.XYZWC, op=A.max)
        # Sqrt table load is eager at the scalar queue head (~22us); both
        # sqrts (this one and the deaths one) use it.
        nc.scalar.sqrt(lmxd[:], lmx[:])
        cin = dram.tile([1, 1], DT)
        cout = dram.tile([1, n_gather], DT)
        nc.scalar.dma_start(cin[:], lmxd[:])
        nc.gpsimd.collective_compute(
            "AllGather", A.bypass, replica_groups=cc_groups,
            ins=[cin.opt()], outs=[cout.opt()],
        )
        nc.sync.dma_start(gsb[:], cout[:])

        # ---- premasked values: on GpSimd (mult is Pool-legal), under the
        # FW window ----
        dut = sb.tile([B, VV], DT)
        nc.gpsimd.tensor_tensor(out=dut[:], in0=d2[:], in1=utsb[:], op=A.mult)

        # ---- Floyd-Warshall min-max closure on d2 (vector only: the Pool
        # engine's TensorTensor has no min/max/is_ge) ----
        M = sb.tile([B, VV], DT)
        M3 = M.rearrange("p (i j) -> p i j", i=V)
        fwt = sb.tile([B, V, V], DT)
        for k in range(V):
            src = d23 if k == 0 else M3
            nc.vector.tensor_tensor(
                out=fwt[:],
                in0=src[:, :, k : k + 1].broadcast_to([B, V, V]),
                in1=src[:, k : k + 1, :].broadcast_to([B, V, V]),
                op=A.max,
            )
            nc.vector.tensor_tensor(out=M3[:], in0=src[:], in1=fwt[:], op=A.min)

        # ---- MST mask + masked upper-tri values (d2 scale) ----
        mk = sb.tile([B, VV], DT)
        nc.vector.tensor_tensor(out=mk[:], in0=M[:], in1=d2[:], op=A.is_ge)
        val = sb.tile([B, VV], DT)
        nc.vector.tensor_tensor(out=val[:], in0=mk[:], in1=dut[:], op=A.mult)

        # ---- extract 24 MST weights^2: 3 rounds of top-8 + match_replace ----
        dsq = sb.tile([B, NT], DT)
        mr1 = sb.tile([B, VV], DT)
        mr2 = sb.tile([B, VV], DT)
        nc.vector.max(dsq[:, 0:8], val[:])
        nc.vector.match_replace(mr1[:], dsq[:, 0:8], val[:], 0.0)
        nc.vector.max(dsq[:, 8:16], mr1[:])
        nc.vector.match_replace(mr2[:], dsq[:, 8:16], mr1[:], 0.0)
        nc.vector.max(dsq[:, 16:24], mr2[:])

        # deaths = sqrt(deaths^2): tiny, Sqrt table already resident
        deaths = sb.tile([B, NT], DT)
        nc.scalar.sqrt(deaths[:], dsq[:])
        # warm ops pinned after the sqrt: pull the SquareExp table reload
        # into the FW/CC window so the tail pays no table switch
        wtmp = sb.tile([1, 2], DT)
        nc.scalar.square(wtmp[:, 0:1], deaths[0:1, 0:1])
        nc.scalar.activation(wtmp[:, 1:2], deaths[0:1, 0:1], ACT.Exp,
                             bias=0.0, scale=-1.0)

        # ---- replicate deaths onto both partition halves (pre-CC) ----
        deaths2 = psum.tile([128, NT], DT)
        nc.tensor.matmul(out=deaths2[:], lhsT=rep128[:], rhs=deaths[:],
                         start=True, stop=True)

        # ---- post-CC: global max -> inv -> normalize ----
        # pin: first post-CC vector op also depends on the last extraction
        # round so the scheduler cannot hoist it into the FW stream
        g8 = sb.tile([1, n_gather], DT)
        nc.vector.tensor_tensor(out=g8[:], in0=gsb[:],
                                in1=dsq[0:1, 16 : 16 + n_gather],
                                op=A.bypass)
        gmx = sb.tile([1, 1], DT)
        nc.vector.tensor_reduce(out=gmx[:], in_=g8[:],
                                axis=mybir.AxisListType.X, op=A.max)
        inv = sb.tile([1, 1], DT)
        nc.vector.reciprocal(inv[:], gmx[:])
        invb = psum.tile([128, 1], DT)
        nc.tensor.matmul(out=invb[:], lhsT=ones1[:], rhs=inv[:],
                         start=True, stop=True)
        dn = sb.tile([128, NT], DT)
        nc.vector.tensor_scalar_mul(dn[:], deaths2[:], invb[:, 0:1])

        # ---- structure element layer on 128 partitions (e-half per h) ----
        S = sb.tile([128, E2], DT)
        ECH = 16
        for ch in range(E2 // ECH):
            e0 = ch * ECH
            t1 = work.tile([128, ECH, NT], DT, tag="t1")
            nc.vector.tensor_tensor(
                out=t1[:],
                in0=dn.unsqueeze(1).broadcast_to([128, ECH, NT]),
                in1=c2p[:, e0 : e0 + ECH].unsqueeze(-1).broadcast_to([128, ECH, NT]),
                op=A.subtract,
            )
            nc.scalar.square(t1[:], t1[:])
            nc.vector.tensor_tensor(
                out=t1[:],
                in0=t1[:],
                in1=ub[:, e0 : e0 + ECH].unsqueeze(-1).broadcast_to([128, ECH, NT]),
                op=A.mult,
            )
            fexp = work.tile([128, ECH, NT], DT, tag="fexp")
            nc.scalar.activation(fexp[:], t1[:], ACT.Exp, bias=0.0, scale=-1.0)
            nc.vector.tensor_reduce(
                out=S[:, e0 : e0 + ECH], in_=fexp[:], axis=mybir.AxisListType.X,
                op=A.add,
            )
        outt = sb.tile([128, E2], DT)
        nc.vector.tensor_tensor(out=outt[:], in0=S[:], in1=Ab[:], op=A.mult)

        # ---- output: two direct DMAs, triggers on different engines so
        # the DIRECT2D processing overlaps ----
        nc.sync.dma_start(out_d[:, 0:E2], outt[0:B])
        nc.scalar.dma_start(out_d[:, E2:E], outt[B:128])

    _split_excess_waits(nc)
    return nc


_CACHE = {}


def _consts():
    # pair matrix: adds partition rows b and b+64 (the two T-halves) and
    # applies the 1/T mean scale
    pairmat = np.zeros((128, B), dtype=np.float32)
    for p in range(128):
        pairmat[p, p % B] = 1.0 / T
    ut = np.triu(np.ones((V, V), dtype=np.float32), k=1).reshape(1, VV)
    rep = np.zeros((B, 128), dtype=np.float32)
    for p in range(128):
        rep[p % B, p] = 1.0
    sel = np.zeros((2, 128), dtype=np.float32)
    sel[0, 0:B] = 1.0
    sel[1, B:128] = 1.0
    return pairmat, np.ascontiguousarray(ut), rep, sel


def _get_program(cc_groups=None, key="nc"):
    if key not in _CACHE:
        _CACHE[key] = _build_program(cc_groups)
    return _CACHE[key]


def _run(x, centres, sharpness, cc_groups=None, key="nc", **run_kwargs):
    nc = _get_program(cc_groups, key)
    xf = np.ascontiguousarray(x.reshape(-1, C, T, V)).astype(np.float32, copy=False)
    n_total = xf.shape[0]
    assert n_total == N_CORES * B, xf.shape
    c1, c2 = centres[:, 0], centres[:, 1]
    s1, s2 = sharpness[:, 0], sharpness[:, 1]
    csH = np.stack(
        [
            np.concatenate([c2[h * E2 : (h + 1) * E2], s2[h * E2 : (h + 1) * E2],
                            s1[h * E2 : (h + 1) * E2], c1[h * E2 : (h + 1) * E2]])
            for h in range(2)
        ],
        axis=0,
    ).astype(np.float32)
    pairmat, ut, rep, sel = _consts()
    in_maps = []
    for i in range(N_CORES):
        xc = xf[i * B : (i + 1) * B]  # [64, C, T, V]
        xt = xc.transpose(0, 1, 3, 2)  # [64, C, V, T]
        xh = np.empty((2, 128, C, V, 32), dtype=np.float32)
        xh[0, 0:64] = xt[..., 0:32]
        xh[0, 64:128] = xt[..., 64:96]
        xh[1, 0:64] = xt[..., 32:64]
        xh[1, 64:128] = xt[..., 96:128]
        in_maps.append(
            {
                "x": np.ascontiguousarray(xh),
                "csH": np.ascontiguousarray(csH),
                "pm": pairmat,
                "ut": ut,
                "rep": rep,
                "sel": sel,
            }
        )
    res = run_bass_kernel_spmd(nc, in_maps, list(range(N_CORES)), **run_kwargs)
    out = np.concatenate([res.results[i]["out"] for i in range(N_CORES)], axis=0)
    return out, res


def kernel(x, centres, sharpness):
    out, _ = _run(np.asarray(x), np.asarray(centres), np.asarray(sharpness))
    return out


# revision 31
# speedup vs baseline: 1.0392x; 1.0092x over previous
"""TopoEncoder Trainium2 kernel v9 (8 NeuronCores, data-parallel over batch).

Measured 87.2-90.2us (baseline v3: 90.6-115us); exec = max(compute
~76.4us, CC end 75-100us) + ~10us tail. Key structure:
  - x host-pretransposed to [2, 128, C, V, t32] (t innermost): the
    T-mean is one axis=X tensor_reduce per c-chunk, and the load is 4
    contiguous-row DMAs on the scalar queue in c01/c2 chunks so each
    reduce starts as its chunk lands. Consts ride the sync queue so
    PE's broadcast inputs are not queued behind the x bulk.
  - Floyd-Warshall min-max closure on d^2 (monotone transform keeps
    closure/mask identical): no [B,625] sqrt; deaths sqrt'd on [B,24]
    after extraction. FW is vector-only: the Pool engine's TensorTensor
    ALU has no min/max/is_ge (arithmetic ops only), so GpSimd instead
    carries the upper-tri premask mult and the local-max reduce.
  - Cross-half add of the two T-halves folded into the PE pair fold
    via PSUM accumulation (two matmuls, start/stop split).
  - The collective AllGathers the local max of d (sqrt'd pre-CC inside
    the Sqrt-table window), so the post-CC path needs no act table:
    vector max + reciprocal + PE [1->128] broadcast only. Table loads
    (SquareExp -> Sqrt -> SquareExp) all hide in the FW window via
    data-pinned warm ops.
  - deaths-replicate matmul pre-CC; the post-CC pin bypass depends on
    the final extraction round so extraction never stalls behind the
    collective on the in-order vector queue.
  - Output: two direct DMAs (e-halves) with triggers on sync+scalar so
    the DIRECT2D processing overlaps.
"""

from contextlib import ExitStack

import numpy as np

import bass_rust
import concourse.bass as bass
import concourse.tile as tile
from concourse import mybir
from concourse.bass_utils import run_bass_kernel_spmd

N_CORES = 8
B = 64          # samples per core
C, T, V, E = 3, 128, 25, 64
VV = V * V
NT = V - 1      # deaths per sample (24)
E2 = E // 2     # e-half per partition group (32)
DT = mybir.dt.float32


def _split_excess_waits(nc, cap=1):
    """The walrus build in this env rejects instructions carrying more than
    ~2 semaphore-wait commands. Move excess waits onto same-engine NOPs
    inserted immediately before the offending instruction."""
    n_split = 0
    for bb in nc.main_func.blocks:
        insts = bb.instructions
        i = 0
        while i < len(insts):
            ins = insts[i]
            si = ins.sync_info
            waits = list(si.on_wait) if si and si.on_wait else []
            if len(waits) > cap:
                extra, keep = waits[:-cap], waits[-cap:]
                ins.sync_info = mybir.SyncInfo(
                    on_wait=keep, on_update=list(si.on_update or [])
                )
                for j, w in enumerate(extra):
                    nop = bass_rust.InstNoOp(
                        name=f"I-wsplit-{n_split}-{j}",
                        engine=ins.engine,
                        sync_info=mybir.SyncInfo(on_wait=[w], on_update=[]),
                    )
                    insts.insert(i, nop)
                    i += 1
                n_split += 1
            i += 1
    return n_split


def _build_program(cc_groups=None):
    A = mybir.AluOpType
    ACT = mybir.ActivationFunctionType
    nc = bass.Bass("TRN2", debug=False, num_devices=N_CORES)
    if cc_groups is None:
        cc_groups = [list(range(N_CORES))]
    n_gather = len(cc_groups[0])

    # x pre-transposed on host: [half, partition, c, v, t32] (t innermost
    # so the T-reduction is a single axis=X tensor_reduce per engine slice)
    x_in = nc.dram_tensor("x", [2, 128, C, V, 32], DT, kind="ExternalInput").ap()
    # csH rows: h -> [c2_h(32) | s2_h(32) | s1_h(32) | c1_h(32)]
    csH_in = nc.dram_tensor("csH", [2, 4 * E2], DT, kind="ExternalInput").ap()
    pm_in = nc.dram_tensor("pm", [128, B], DT, kind="ExternalInput").ap()
    ut_in = nc.dram_tensor("ut", [1, VV], DT, kind="ExternalInput").ap()
    rep_in = nc.dram_tensor("rep", [B, 128], DT, kind="ExternalInput").ap()
    sel_in = nc.dram_tensor("sel", [2, 128], DT, kind="ExternalInput").ap()
    out_d = nc.dram_tensor("out", [B, E], DT, kind="ExternalOutput").ap()

    with tile.TileContext(nc, num_cores=N_CORES) as tc, ExitStack() as ctx:
        sb = ctx.enter_context(tc.tile_pool(name="sb", bufs=1))
        work = ctx.enter_context(tc.tile_pool(name="work", bufs=2))
        psum = ctx.enter_context(tc.tile_pool(name="psum", bufs=1, space="PSUM"))
        dram = ctx.enter_context(tc.tile_pool(name="dram", bufs=1, space="DRAM"))

        ones1 = sb.tile([1, 128], DT)
        nc.vector.memset(ones1[:], 1.0)

        # ---- x DMA on scalar in c01/c2 chunks (reduces start per chunk);
        # consts on sync so PE's inputs are not queued behind the x bulk
        xa = sb.tile([128, C, V, T // 4], DT)
        xb = sb.tile([128, C, V, T // 4], DT)
        nc.scalar.dma_start(xa[:, 0:2], x_in[0, :, 0:2])
        nc.scalar.dma_start(xa[:, 2], x_in[0, :, 2])
        nc.scalar.dma_start(xb[:, 0:2], x_in[1, :, 0:2])
        nc.scalar.dma_start(xb[:, 2], x_in[1, :, 2])

        # ---- small constant loads ----
        pm_t = sb.tile([128, B], DT)
        nc.sync.dma_start(pm_t[:], pm_in[:])
        csh = sb.tile([2, 4 * E2], DT)
        nc.sync.dma_start(csh[:], csH_in[:])
        utrow = sb.tile([1, VV], DT)
        nc.sync.dma_start(utrow[:], ut_in[:])
        rep128 = sb.tile([B, 128], DT)
        nc.sync.dma_start(rep128[:], rep_in[:])
        sel2 = sb.tile([2, 128], DT)
        nc.sync.dma_start(sel2[:], sel_in[:])
        lmx = sb.tile([1, 1], DT)
        lmxd = sb.tile([1, 1], DT)
        gsb = sb.tile([1, n_gather], DT)

        # ---- PE partition-broadcasts ----
        utb = psum.tile([B, VV], DT)
        nc.tensor.matmul(out=utb[:, 0:512], lhsT=ones1[:, 0:B],
                         rhs=utrow[:, 0:512], start=True, stop=True)
        nc.tensor.matmul(out=utb[:, 512:VV], lhsT=ones1[:, 0:B],
                         rhs=utrow[:, 512:VV], start=True, stop=True)
        # GpSimd cannot read PSUM: bounce the upper-tri broadcast to SBUF
        # early, while the vector queue is idle
        utsb = sb.tile([B, VV], DT)
        nc.vector.tensor_copy(utsb[:], utb[:])
        # per-half params onto 128 partitions: partition p gets half p//64
        prm2 = psum.tile([128, 4, E2], DT)
        nc.tensor.matmul(out=prm2[:], lhsT=sel2[:], rhs=csh[:],
                         start=True, stop=True)
        prm = sb.tile([128, 4, E2], DT)
        nc.vector.tensor_copy(prm[:], prm2[:])
        c2p = prm[:, 0, :]

        # derived parameter tiles: ub = s2^2, Ab = exp(-(s1*c1)^2)
        ub = sb.tile([128, E2], DT)
        nc.scalar.square(ub[:], prm[:, 1, :])
        m1t = sb.tile([128, E2], DT)
        nc.vector.tensor_tensor(out=m1t[:], in0=prm[:, 2, :], in1=prm[:, 3, :],
                                op=A.mult)
        nc.scalar.square(m1t[:], m1t[:])
        Ab = sb.tile([128, E2], DT)
        nc.scalar.activation(Ab[:], m1t[:], ACT.Exp, bias=0.0, scale=-1.0)

        # ---- mean over T: single axis=X reductions (DVE: c0-c1 + part of
        # c2, GpSimd: rest of c2 -- balanced by engine rates), cross-half
        # add, then PE pair-matrix fold ----
        ra = sb.tile([128, C, V], DT)
        rb = sb.tile([128, C, V], DT)
        for xh, rh in ((xa, ra), (xb, rb)):
            nc.vector.tensor_reduce(out=rh[:, 0:2], in_=xh[:, 0:2],
                                    axis=mybir.AxisListType.X, op=A.add)
            nc.vector.tensor_reduce(out=rh[:, 2], in_=xh[:, 2],
                                    axis=mybir.AxisListType.X, op=A.add)
        # cross-half add folded into the PE pair fold via PSUM accumulation
        ps_xm = psum.tile([B, C, V], DT)
        nc.tensor.matmul(out=ps_xm[:], lhsT=pm_t[:], rhs=ra[:],
                         start=True, stop=False)
        nc.tensor.matmul(out=ps_xm[:], lhsT=pm_t[:], rhs=rb[:],
                         start=False, stop=True)
        xm = sb.tile([B, C, V], DT)
        nc.vector.tensor_copy(xm[:], ps_xm[:])

        # ---- squared distance matrix (no sqrt: FW closure is monotone);
        # all subtracts on V (GpSimd is 3x slower and gated the adds) ----
        df = sb.tile([B, C, V, V], DT)
        xmb_i = xm.unsqueeze(-1).broadcast_to([B, C, V, V])
        xmb_j = xm.unsqueeze(2).broadcast_to([B, C, V, V])
        nc.vector.tensor_tensor(
            out=df[:, 0:2], in0=xmb_i[:, 0:2], in1=xmb_j[:, 0:2], op=A.subtract
        )
        nc.vector.tensor_tensor(
            out=df[:, 2], in0=xmb_i[:, 2], in1=xmb_j[:, 2], op=A.subtract
        )
        nc.scalar.square(df[:, 0:2], df[:, 0:2])
        nc.vector.tensor_tensor(out=df[:, 2], in0=df[:, 2], in1=df[:, 2], op=A.mult)
        d2 = sb.tile([B, VV], DT)
        d23 = d2.rearrange("p (i j) -> p i j", i=V)
        nc.vector.tensor_tensor(out=d23[:], in0=df[:, 0], in1=df[:, 1], op=A.add)
        nc.vector.tensor_tensor(out=d23[:], in0=d23[:], in1=df[:, 2], op=A.add)

        # ---- local max(d2) -> sqrt -> AllGather of local max(d) ----
        nc.gpsimd.tensor_reduce(out=lmx[:], in_=d2[:],
                                axis=mybir.AxisListType.XYZWC, op=A.max)
        # Sqrt table load is eager at the scalar queue head (~22us); both
        # sqrts (this one and the deaths one) use it.
        nc.scalar.sqrt(lmxd[:], lmx[:])
        cin = dram.tile([1, 1], DT)
        cout = dram.tile([1, n_gather], DT)
        nc.scalar.dma_start(cin[:], lmxd[:])
        nc.gpsimd.collective_compute(
            "AllGather", A.bypass, replica_groups=cc_groups,
            ins=[cin.opt()], outs=[cout.opt()],
        )
        nc.sync.dma_start(gsb[:], cout[:])

        # ---- premasked values: on GpSimd (mult is Pool-legal), under the
        # FW window ----
        dut = sb.tile([B, VV], DT)
        nc.gpsimd.tensor_tensor(out=dut[:], in0=d2[:], in1=utsb[:], op=A.mult)

        # ---- Floyd-Warshall min-max closure on d2 (vector only: the Pool
        # engine's TensorTensor has no min/max/is_ge) ----
        M = sb.tile([B, VV], DT)
        M3 = M.rearrange("p (i j) -> p i j", i=V)
        fwt = sb.tile([B, V, V], DT)
        for k in range(V):
            src = d23 if k == 0 else M3
            nc.vector.tensor_tensor(
                out=fwt[:],
                in0=src[:, :, k : k + 1].broadcast_to([B, V, V]),
                in1=src[:, k : k + 1, :].broadcast_to([B, V, V]),
                op=A.max,
            )
            nc.vector.tensor_tensor(out=M3[:], in0=src[:], in1=fwt[:], op=A.min)

        # ---- MST mask + masked upper-tri values (d2 scale) ----
        mk = sb.tile([B, VV], DT)
        nc.vector.tensor_tensor(out=mk[:], in0=M[:], in1=d2[:], op=A.is_ge)
        val = sb.tile([B, VV], DT)
        nc.vector.tensor_tensor(out=val[:], in0=mk[:], in1=dut[:], op=A.mult)

        # ---- extract 24 MST weights^2: 3 rounds of top-8 + match_replace ----
        dsq = sb.tile([B, NT], DT)
        mr1 = sb.tile([B, VV], DT)
        mr2 = sb.tile([B, VV], DT)
        nc.vector.max(dsq[:, 0:8], val[:])
        nc.vector.match_replace(mr1[:], dsq[:, 0:8], val[:], 0.0)
        nc.vector.max(dsq[:, 8:16], mr1[:])
        nc.vector.match_replace(mr2[:], dsq[:, 8:16], mr1[:], 0.0)
        nc.vector.max(dsq[:, 16:24], mr2[:])

        # deaths = sqrt(deaths^2): tiny, Sqrt table already resident
        deaths = sb.tile([B, NT], DT)
        nc.scalar.sqrt(deaths[:], dsq[:])
        # warm ops pinned after the sqrt: pull the SquareExp table reload
        # into the FW/CC window so the tail pays no table switch
        wtmp = sb.tile([1, 2], DT)
        nc.scalar.square(wtmp[:, 0:1], deaths[0:1, 0:1])
        nc.scalar.activation(wtmp[:, 1:2], deaths[0:1, 0:1], ACT.Exp,
                             bias=0.0, scale=-1.0)

        # ---- replicate deaths onto both partition halves (pre-CC) ----
        deaths2 = psum.tile([128, NT], DT)
        nc.tensor.matmul(out=deaths2[:], lhsT=rep128[:], rhs=deaths[:],
                         start=True, stop=True)

        # ---- post-CC: global max -> inv -> normalize ----
        # pin: first post-CC vector op also depends on the last extraction
        # round so the scheduler cannot hoist it into the FW stream
        g8 = sb.tile([1, n_gather], DT)
        nc.vector.tensor_tensor(out=g8[:], in0=gsb[:],
                                in1=dsq[0:1, 16 : 16 + n_gather],
                                op=A.bypass)
        gmx = sb.tile([1, 1], DT)
        nc.vector.tensor_reduce(out=gmx[:], in_=g8[:],
                                axis=mybir.AxisListType.X, op=A.max)
        inv = sb.tile([1, 1], DT)
        nc.vector.reciprocal(inv[:], gmx[:])
        invb = psum.tile([128, 1], DT)
        nc.tensor.matmul(out=invb[:], lhsT=ones1[:], rhs=inv[:],
                         start=True, stop=True)
        dn = sb.tile([128, NT], DT)
        nc.vector.tensor_scalar_mul(dn[:], deaths2[:], invb[:, 0:1])

        # ---- structure element layer on 128 partitions (e-half per h) ----
        S = sb.tile([128, E2], DT)
        ECH = 16
        for ch in range(E2 // ECH):
            e0 = ch * ECH
            t1 = work.tile([128, ECH, NT], DT, tag="t1")
            nc.vector.tensor_tensor(
                out=t1[:],
                in0=dn.unsqueeze(1).broadcast_to([128, ECH, NT]),
                in1=c2p[:, e0 : e0 + ECH].unsqueeze(-1).broadcast_to([128, ECH, NT]),
                op=A.subtract,
            )
            nc.scalar.square(t1[:], t1[:])
            nc.vector.tensor_tensor(
                out=t1[:],
                in0=t1[:],
                in1=ub[:, e0 : e0 + ECH].unsqueeze(-1).broadcast_to([128, ECH, NT]),
                op=A.mult,
            )
            fexp = work.tile([128, ECH, NT], DT, tag="fexp")
            nc.scalar.activation(fexp[:], t1[:], ACT.Exp, bias=0.0, scale=-1.0)
            nc.vector.tensor_reduce(
                out=S[:, e0 : e0 + ECH], in_=fexp[:], axis=mybir.AxisListType.X,
                op=A.add,
            )
        outt = sb.tile([128, E2], DT)
        nc.vector.tensor_tensor(out=outt[:], in0=S[:], in1=Ab[:], op=A.mult)

        # ---- output: two direct DMAs, triggers on different engines so
        # the DIRECT2D processing overlaps ----
        nc.sync.dma_start(out_d[:, 0:E2], outt[0:B])
        nc.scalar.dma_start(out_d[:, E2:E], outt[B:128])

    _split_excess_waits(nc)
    return nc


_CACHE = {}


def _consts():
    # pair matrix: adds partition rows b and b+64 (the two T-halves) and
    # applies the 1/T mean scale
    pairmat = np.zeros((128, B), dtype=np.float32)
    for p in range(128):
        pairmat[p, p % B] = 1.0 / T
    ut = np.triu(np.ones((V, V), dtype=np.float32), k=1).reshape(1, VV)
    rep = np.zeros((B, 128), dtype=np.float32)
    for p in range(128):
        rep[p % B, p] = 1.0
    sel = np.zeros((2, 128), dtype=np.float32)
    sel[0, 0:B] = 1.0
    sel[1, B:128] = 1.0
    return pairmat, np.ascontiguousarray(ut), rep, sel


def _get_program(cc_groups=None, key="nc"):
    if key not in _CACHE:
        _CACHE[key] = _build_program(cc_groups)
    return _CACHE[key]


def _run(x, centres, sharpness, cc_groups=None, key="nc", **run_kwargs):
    nc = _get_program(cc_groups, key)
    xf = np.ascontiguousarray(x.reshape(-1, C, T, V)).astype(np.float32, copy=False)
    n_total = xf.shape[0]
    assert n_total == N_CORES * B, xf.shape
    c1, c2 = centres[:, 0], centres[:, 1]
    s1, s2 = sharpness[:, 0], sharpness[:, 1]
    csH = np.stack(
        [
            np.concatenate([c2[h * E2 : (h + 1) * E2], s2[h * E2 : (h + 1) * E2],
                            s1[h * E2 : (h + 1) * E2], c1[h * E2 : (h + 1) * E2]])
            for h in range(2)
        ],
        axis=0,
    ).astype(np.float32)
    pairmat, ut, rep, sel = _consts()
    in_maps = []
    for i in range(N_CORES):
        xc = xf[i * B : (i + 1) * B]  # [64, C, T, V]
        xt = xc.transpose(0, 1, 3, 2)  # [64, C, V, T]
        xh = np.empty((2, 128, C, V, 32), dtype=np.float32)
        xh[0, 0:64] = xt[..., 0:32]
        xh[0, 64:128] = xt[..., 64:96]
        xh[1, 0:64] = xt[..., 32:64]
        xh[1, 64:128] = xt[..., 96:128]
        in_maps.append(
            {
                "x": np.ascontiguousarray(xh),
                "csH": np.ascontiguousarray(csH),
                "pm": pairmat,
                "ut": ut,
                "rep": rep,
                "sel": sel,
            }
        )
    res = run_bass_kernel_spmd(nc, in_maps, list(range(N_CORES)), **run_kwargs)
    out = np.concatenate([res.results[i]["out"] for i in range(N_CORES)], axis=0)
    return out, res


def kernel(x, centres, sharpness):
    out, _ = _run(np.asarray(x), np.asarray(centres), np.asarray(sharpness))
    return out
